# revision 9
# baseline (speedup 1.0000x reference)
"""Trainium2 Bass kernel for one Tacotron2-style decoder iteration (B=128, T=1024).

Sharding (8 NeuronCores):
  - LSTM gate GEMMs: tensor-parallel over the 4096 gate dim (each core computes
    all 128 batch rows for its 512-gate slice; full batch on PE partitions).
  - Attention (conv / energies / softmax / context): data-parallel over batch
    (16 rows per core); memory + processed_memory sharded on batch.
  - 3 small AllGathers stitch the two shardings (ah.T, ctx, dh.T).
Matmuls run as float32r (TF32-like, full PE rate at N=512, ~1e-4 rel err).
"""
import os
import numpy as np
from contextlib import ExitStack

import concourse.bass as bass
import concourse.tile as tile
from concourse import bacc, mybir
from concourse.bass_utils import run_bass_kernel_spmd

F32 = mybir.dt.float32
R32 = mybir.dt.float32r
AF = mybir.ActivationFunctionType
AX = mybir.AxisListType
ALU = mybir.AluOpType

B, T = 128, 1024
NMEL, PRE, ENC = 80, 256, 512
HID, ATT_DIM = 1024, 128
KS, PAD = 31, 15
NC = 8
BS = B // NC          # 16 batch rows per core
TP = 4 * HID // NC    # 512 gates per core

M1_HEX = 'c97c08e34c7fb118f54fc3d10eb6048fc55d94ce2f655042235dcd09c68c8ec2'
M2_HEX = '890b3c4b87f818705fa117651490d99f1ff387ed1a502fb078888875ce3aa467'
_m1 = np.unpackbits(np.frombuffer(bytes.fromhex(M1_HEX), np.uint8))[:PRE].astype(np.float32)
_m2 = np.unpackbits(np.frombuffer(bytes.fromhex(M2_HEX), np.uint8))[:PRE].astype(np.float32)


def _build():
    PL = int(os.environ.get('KPL', '9'))
    nc = bacc.Bacc("TRN2", target_bir_lowering=False, debug=False, num_devices=NC)

    def di(name, shape, dt=R32):
        return nc.dram_tensor(name, shape, dt, kind="ExternalInput")

    def do(name, shape, dt=F32):
        return nc.dram_tensor(name, shape, dt, kind="ExternalOutput")

    pre1T = di("pre1T", [NMEL, PRE])
    pre2T = di("pre2T", [PRE, PRE])
    dec_inT = di("dec_inT", [NMEL, B])
    ctxaT = di("ctxaT", [ENC, B])
    hT_att = di("hT_att", [HID, B])
    hT_dec = di("hT_dec", [HID, B])
    W_qT = di("W_qT", [HID, ATT_DIM])
    conv_lhsT = di("conv_lhsT", [62, ATT_DIM])
    v_col = di("v_col", [ATT_DIM, 1])
    WpgT = di("WpgT", [HID + ENC, 82])
    b_pg = di("b_pg", [1, 82])
    identF = di("identF", [128, 128], F32)
    ones_in = di("ones_in", [1, 128])
    identR = di("identR", [128, 128])
    bsel = di("bsel", [B, BS])
    Wih_attT = di("Wih_attT", [PRE + ENC, TP])
    Whh_attT = di("Whh_attT", [HID, TP])
    b_att = di("b_att", [1, TP])
    Wih_decT = di("Wih_decT", [HID + ENC, TP])
    Whh_decT = di("Whh_decT", [HID, TP])
    b_dec = di("b_dec", [1, TP])
    catt = di("catt", [B, 128], F32)
    cdec = di("cdec", [B, 128], F32)
    awc_pad = di("awc_pad", [BS, 2, T + 2 * PAD])
    awc_in = di("awc_in", [BS, T], F32)
    pm_t = di("pm_t", [ATT_DIM, BS, T])
    mem = di("mem", [BS, T, ENC])

    o_ah = do("o_ah", [B, 128])
    o_ac = do("o_ac", [B, 128])
    o_dh = do("o_dh", [B, 128])
    o_dc = do("o_dc", [B, 128])
    o_aw = do("o_aw", [BS, T])
    o_awcum = do("o_awcum", [BS, T])
    o_ctx = do("o_ctx", [BS, ENC])
    o_hd = do("o_hd", [B, 82])

    ag1_in = nc.dram_tensor("ag1_in", [128, B], F32)
    ag1_out = nc.dram_tensor("ag1_out", [HID, B], F32, addr_space="Shared")
    ag2_in = nc.dram_tensor("ag2_in", [BS, ENC], F32)
    ag2_out = nc.dram_tensor("ag2_out", [B, ENC], F32, addr_space="Shared")
    ag3_in = nc.dram_tensor("ag3_in", [128, B], F32)
    ag3_out = nc.dram_tensor("ag3_out", [HID, B], F32, addr_space="Shared")

    with tile.TileContext(nc) as tc, ExitStack() as ctx:
        wp = ctx.enter_context(tc.tile_pool(name="wp", bufs=1))
        sp1 = ctx.enter_context(tc.tile_pool(name="sp1", bufs=1))
        ap_ = ctx.enter_context(tc.tile_pool(name="ap", bufs=2))
        pmp = ctx.enter_context(tc.tile_pool(name="pmp", bufs=2))
        pcp = ctx.enter_context(tc.tile_pool(name="pcp", bufs=2))
        thp = ctx.enter_context(tc.tile_pool(name="thp", bufs=2))
        stg = ctx.enter_context(tc.tile_pool(name="stg", bufs=3))
        ring = ctx.enter_context(tc.tile_pool(name="ring", bufs=12))
        ps_s = ctx.enter_context(tc.tile_pool(name="ps_s", bufs=2, space="PSUM"))
        ps_g = ctx.enter_context(tc.tile_pool(name="ps_g", bufs=1, space="PSUM"))
        ps_t = ctx.enter_context(tc.tile_pool(name="ps_t", bufs=2, space="PSUM"))
        ps_v = ctx.enter_context(tc.tile_pool(name="ps_v", bufs=2, space="PSUM"))

        # ---------- inputs (phase-1 critical first, high priority) ----------
        with tc.high_priority():
            identF_sb = wp.tile([128, 128], F32, tag="identF")
            nc.sync.dma_start(identF_sb[:], identF[:])
            identR_sb = wp.tile([128, 128], R32, tag="identR")
            nc.sync.dma_start(identR_sb[:], identR[:])
            pre1_sb = wp.tile([NMEL, PRE], R32, tag="pre1")
            nc.sync.dma_start(pre1_sb[:], pre1T[:])
            pre2_sb = wp.tile([128, 2, PRE], R32, tag="pre2")
            nc.sync.dma_start(pre2_sb[:], pre2T.ap().rearrange("(c k) n -> k c n", k=128))
            dinT_sb = wp.tile([NMEL, B], R32, tag="dinT")
            nc.sync.dma_start(dinT_sb[:], dec_inT[:])
            ctxaT_sb = wp.tile([128, 4, B], R32, tag="ctxaT")
            nc.sync.dma_start(ctxaT_sb[:], ctxaT.ap().rearrange("(c k) n -> k c n", k=128))
            hTa_sb = wp.tile([128, 8, B], R32, tag="hTa")
            nc.sync.dma_start(hTa_sb[:], hT_att.ap().rearrange("(c k) n -> k c n", k=128))
            wia_sb = wp.tile([128, 6, TP], R32, tag="wia")
            nc.sync.dma_start(wia_sb[:], Wih_attT.ap().rearrange("(c k) n -> k c n", k=128))
            wha_sb = wp.tile([128, 8, TP], R32, tag="wha")
            nc.sync.dma_start(wha_sb[:], Whh_attT.ap().rearrange("(c k) n -> k c n", k=128))
            ba_sb = wp.tile([1, TP], R32, tag="ba")
            nc.sync.dma_start(ba_sb[:], b_att[:])
            catt_sb = wp.tile([B, 128], F32, tag="catt")
            nc.sync.dma_start(catt_sb[:], catt[:])
            wq_sb = wp.tile([128, 8, ATT_DIM], R32, tag="wq")
            nc.sync.dma_start(wq_sb[:], W_qT.ap().rearrange("(c k) n -> k c n", k=128))
            conv_sb = wp.tile([62, ATT_DIM], R32, tag="convw")
            nc.sync.dma_start(conv_sb[:], conv_lhsT[:])
            v_sb = wp.tile([ATT_DIM, 1], R32, tag="v")
            nc.sync.dma_start(v_sb[:], v_col[:])
            ones_sb = wp.tile([1, 128], R32, tag="ones")
            nc.sync.dma_start(ones_sb[:], ones_in[:])

        hTd_sb = wp.tile([128, 8, B], R32, tag="hTd")
        nc.sync.dma_start(hTd_sb[:], hT_dec.ap().rearrange("(c k) n -> k c n", k=128))
        wid_sb = wp.tile([128, 12, TP], R32, tag="wid")
        nc.sync.dma_start(wid_sb[:], Wih_decT.ap().rearrange("(c k) n -> k c n", k=128))
        whd_sb = wp.tile([128, 8, TP], R32, tag="whd")
        nc.sync.dma_start(whd_sb[:], Whh_decT.ap().rearrange("(c k) n -> k c n", k=128))
        bd_sb = wp.tile([1, TP], R32, tag="bd")
        nc.sync.dma_start(bd_sb[:], b_dec[:])
        cdec_sb = wp.tile([B, 128], F32, tag="cdec")
        nc.sync.dma_start(cdec_sb[:], cdec[:])
        wpg_sb = wp.tile([128, 12, 82], R32, tag="wpg")
        nc.sync.dma_start(wpg_sb[:], WpgT.ap().rearrange("(c k) n -> k c n", k=128))
        bpg_sb = wp.tile([1, 82], R32, tag="bpg")
        nc.sync.dma_start(bpg_sb[:], b_pg[:])
        awcin_sb = wp.tile([BS, T], F32, tag="awcin")
        nc.sync.dma_start(awcin_sb[:], awc_in[:])
        bsel_sb = wp.tile([B, BS], R32, tag="bsel")
        nc.sync.dma_start(bsel_sb[:], bsel[:])

        def lstm_tail(g, c_sb, hname, cname, hout, cout):
            si = ap_.tile([B, 128], F32, tag="si")
            nc.scalar.activation(si[:], g[:, 0:128], AF.Sigmoid)
            sf = ap_.tile([B, 128], F32, tag="sf")
            nc.scalar.activation(sf[:], g[:, 128:256], AF.Sigmoid)
            tg = ap_.tile([B, 128], F32, tag="tg")
            nc.scalar.activation(tg[:], g[:, 256:384], AF.Tanh)
            so = ap_.tile([B, 128], F32, tag="so")
            nc.scalar.activation(so[:], g[:, 384:512], AF.Sigmoid)
            t1 = ap_.tile([B, 128], F32, tag="t1")
            nc.vector.tensor_mul(t1[:], si[:], tg[:])
            t2 = ap_.tile([B, 128], F32, tag="t2")
            nc.vector.tensor_mul(t2[:], sf[:], c_sb[:])
            c2t = ap_.tile([B, 128], F32, tag=cname)
            nc.vector.tensor_add(c2t[:], t1[:], t2[:])
            tc2 = ap_.tile([B, 128], F32, tag="tc2")
            nc.scalar.activation(tc2[:], c2t[:], AF.Tanh)
            h2t = ap_.tile([B, 128], F32, tag=hname)
            nc.vector.tensor_mul(h2t[:], so[:], tc2[:])
            nc.sync.dma_start(hout.ap(), h2t[:])
            nc.sync.dma_start(cout.ap(), c2t[:])
            return h2t

        # ---------- phase 1: prenet + attention LSTM + AG1 ----------
        x1_sb = ap_.tile([128, 2, B], R32, tag="x1")
        for c in range(2):
            xp = ps_t.tile([128, B], F32, tag="tr")
            nc.tensor.matmul(xp[:], pre1_sb[:, c * 128:(c + 1) * 128], dinT_sb[:],
                             start=True, stop=True)
            nc.vector.tensor_relu(x1_sb[:, c, :], xp[:])
        x2_sb = ap_.tile([128, 2, B], R32, tag="x2")
        for c in range(2):
            xp = ps_t.tile([128, B], F32, tag="tr")
            for c2 in range(2):
                nc.tensor.matmul(xp[:], pre2_sb[:, c2, c * 128:(c + 1) * 128],
                                 x1_sb[:, c2, :], start=(c2 == 0), stop=(c2 == 1))
            nc.vector.tensor_relu(x2_sb[:, c, :], xp[:])

        g1 = ps_g.tile([B, TP], F32, tag="g")
        nc.tensor.matmul(g1[:], ones_sb[:], ba_sb[:], start=True, stop=False)
        for c in range(2):
            nc.tensor.matmul(g1[:], x2_sb[:, c, :], wia_sb[:, c, :], start=False, stop=False)
        for c in range(4):
            nc.tensor.matmul(g1[:], ctxaT_sb[:, c, :], wia_sb[:, 2 + c, :], start=False, stop=False)
        for c in range(8):
            nc.tensor.matmul(g1[:], hTa_sb[:, c, :], wha_sb[:, c, :], start=False,
                             stop=(c == 7))

        ah_sb = lstm_tail(g1, catt_sb, "ah", "ac", o_ah, o_ac)

        ahT_ps = ps_t.tile([128, B], F32, tag="tr")
        nc.tensor.transpose(ahT_ps[:], ah_sb[:], identF_sb[:])
        ahT_sb = sp1.tile([128, B], F32, tag="ahT")
        nc.vector.tensor_copy(ahT_sb[:], ahT_ps[:])
        nc.sync.dma_start(ag1_in.ap(), ahT_sb[:])
        nc.gpsimd.collective_compute(
            "AllGather", ALU.bypass, ins=[ag1_in.ap()], outs=[ag1_out.ap()],
            replica_groups=[list(range(NC))])
        ahTg_f = sp1.tile([128, 8, B], F32, tag="ahTgf")
        nc.sync.dma_start(ahTg_f[:], ag1_out.ap().rearrange("(c k) n -> k c n", k=128))
        ahTg_sb = sp1.tile([128, 8, B], R32, tag="ahTg")
        nc.vector.tensor_copy(ahTg_sb[:], ahTg_f[:])

        if PL >= 2:
            # ---------- phase 2: q + energies ----------
            q_ps = ps_t.tile([128, B], F32, tag="tr")
            for c in range(8):
                nc.tensor.matmul(q_ps[:], wq_sb[:, c, :], ahTg_sb[:, c, :],
                                 start=(c == 0), stop=(c == 7))
            qT_sb = sp1.tile([128, B], F32, tag="qT")
            nc.vector.tensor_copy(qT_sb[:], q_ps[:])
            qB_ps = ps_t.tile([B, 128], F32, tag="tr")
            nc.tensor.transpose(qB_ps[:], qT_sb[:], identF_sb[:])
            qB_sb = sp1.tile([B, 128], R32, tag="qB")
            nc.vector.tensor_copy(qB_sb[:], qB_ps[:])
            qown_ps = ps_t.tile([BS, 128], F32, tag="tr")
            nc.tensor.matmul(qown_ps[:], bsel_sb[:], qB_sb[:], start=True, stop=True)
            qown_sb = sp1.tile([BS, 128], F32, tag="qown")
            nc.vector.tensor_copy(qown_sb[:], qown_ps[:])
            qTown_ps = ps_t.tile([128, BS], F32, tag="tr")
            nc.tensor.transpose(qTown_ps[:], qown_sb[:], identF_sb[0:BS, 0:BS])
            qTown_sb = sp1.tile([128, BS], F32, tag="qTown")
            nc.vector.tensor_copy(qTown_sb[:], qTown_ps[:])

            e_sb = sp1.tile([BS, T], F32, tag="e")
            for b in range(BS):
                patch_sb = pcp.tile([62, T], R32, tag="patch")
                nc.sync.dma_start(
                    patch_sb[:],
                    bass.AP(awc_pad, b * 2 * (T + 2 * PAD),
                            [[T + 2 * PAD, 2], [1, KS], [1, T]]))
                pm_sb = pmp.tile([ATT_DIM, T], R32, tag="pm")
                nc.sync.dma_start(pm_sb[:], pm_t.ap()[:, b, :])
                for h in range(2):
                    sl = slice(h * 512, (h + 1) * 512)
                    s_ps = ps_s.tile([ATT_DIM, 512], F32, tag="s")
                    nc.tensor.matmul(s_ps[:], conv_sb[:], patch_sb[:, sl],
                                     start=True, stop=False)
                    nc.tensor.matmul(s_ps[:], identR_sb[:], pm_sb[:, sl],
                                     start=False, stop=True)
                    th_sb = thp.tile([ATT_DIM, 512], R32, tag="th")
                    nc.scalar.activation(th_sb[:], s_ps[:], AF.Tanh,
                                         bias=qTown_sb[:, b:b + 1], scale=1.0)
                    e_ps = ps_v.tile([1, 512], F32, tag="vec")
                    nc.tensor.matmul(e_ps[:], v_sb[:], th_sb[:], start=True, stop=True)
                    e_st = stg.tile([1, 512], F32, tag="stg")
                    nc.vector.tensor_copy(e_st[:], e_ps[:])
                    nc.sync.dma_start(e_sb[b:b + 1, sl], e_st[:])

        if PL >= 3:
            # ---------- phase 3: softmax + awT ----------
            mx = sp1.tile([BS, 1], F32, tag="mx")
            nc.vector.tensor_reduce(mx[:], e_sb[:], AX.X, op=ALU.max)
            mneg = sp1.tile([BS, 1], F32, tag="mneg")
            nc.vector.tensor_scalar_mul(mneg[:], mx[:], -1.0)
            ex_sb = sp1.tile([BS, T], F32, tag="ex")
            nc.scalar.activation(ex_sb[:], e_sb[:], AF.Exp, bias=mneg[:], scale=1.0)
            ssum = sp1.tile([BS, 1], F32, tag="ssum")
            nc.vector.tensor_reduce(ssum[:], ex_sb[:], AX.X, op=ALU.add)
            rsum = sp1.tile([BS, 1], F32, tag="rsum")
            nc.vector.reciprocal(rsum[:], ssum[:])
            aw_sb = sp1.tile([BS, T], F32, tag="aw")
            nc.vector.tensor_scalar_mul(aw_sb[:], ex_sb[:], rsum[:])
            nc.sync.dma_start(o_aw.ap(), aw_sb[:])
            awcum_sb = sp1.tile([BS, T], F32, tag="awcum")
            nc.vector.tensor_add(awcum_sb[:], aw_sb[:], awcin_sb[:])
            nc.sync.dma_start(o_awcum.ap(), awcum_sb[:])

            awT_sb = sp1.tile([128, 8, BS], R32, tag="awT")
            for t8 in range(8):
                awT_ps = ps_t.tile([128, BS], F32, tag="tr")
                nc.tensor.transpose(awT_ps[:], aw_sb[:, t8 * 128:(t8 + 1) * 128],
                                    identF_sb[0:BS, 0:BS])
                nc.vector.tensor_copy(awT_sb[:, t8, :], awT_ps[:])

        if PL >= 4:
            # ---------- phase 4: context bmm + AG2 ----------
            ctx_sb = sp1.tile([BS, ENC], F32, tag="ctx")
            for b in range(BS):
                c_ps = ps_v.tile([1, ENC], F32, tag="vec")
                for t8 in range(8):
                    m_sb = ring.tile([128, ENC], R32, tag="mem")
                    nc.sync.dma_start(m_sb[:], mem.ap()[b, t8 * 128:(t8 + 1) * 128, :])
                    nc.tensor.matmul(c_ps[:], awT_sb[:, t8, b:b + 1], m_sb[:],
                                     start=(t8 == 0), stop=(t8 == 7))
                c_st = stg.tile([1, ENC], F32, tag="stg")
                nc.vector.tensor_copy(c_st[:], c_ps[:])
                nc.sync.dma_start(ctx_sb[b:b + 1, :], c_st[:])
            nc.sync.dma_start(o_ctx.ap(), ctx_sb[:])

            nc.sync.dma_start(ag2_in.ap(), ctx_sb[:])
            nc.gpsimd.collective_compute(
                "AllGather", ALU.bypass, ins=[ag2_in.ap()], outs=[ag2_out.ap()],
                replica_groups=[list(range(NC))])
            ctxg_sb = sp1.tile([B, ENC], F32, tag="ctxg")
            nc.sync.dma_start(ctxg_sb[:], ag2_out.ap())
            ctxT_sb = sp1.tile([128, 4, B], R32, tag="ctxT")
            for c in range(4):
                cT_ps = ps_t.tile([128, B], F32, tag="tr")
                nc.tensor.transpose(cT_ps[:], ctxg_sb[:, c * 128:(c + 1) * 128], identF_sb[:])
                nc.vector.tensor_copy(ctxT_sb[:, c, :], cT_ps[:])

        if PL >= 5:
            # ---------- phase 5: decoder LSTM + AG3 ----------
            g2 = ps_g.tile([B, TP], F32, tag="g")
            nc.tensor.matmul(g2[:], ones_sb[:], bd_sb[:], start=True, stop=False)
            for c in range(8):
                nc.tensor.matmul(g2[:], hTd_sb[:, c, :], whd_sb[:, c, :], start=False, stop=False)
            for c in range(8):
                nc.tensor.matmul(g2[:], ahTg_sb[:, c, :], wid_sb[:, c, :], start=False, stop=False)
            for c in range(4):
                nc.tensor.matmul(g2[:], ctxT_sb[:, c, :], wid_sb[:, 8 + c, :], start=False,
                                 stop=(c == 3))
            dh_sb = lstm_tail(g2, cdec_sb, "dh", "dc", o_dh, o_dc)

            dhT_ps = ps_t.tile([128, B], F32, tag="tr")
            nc.tensor.transpose(dhT_ps[:], dh_sb[:], identF_sb[:])
            dhT_sb = sp1.tile([128, B], F32, tag="dhT")
            nc.vector.tensor_copy(dhT_sb[:], dhT_ps[:])
            nc.sync.dma_start(ag3_in.ap(), dhT_sb[:])
            nc.gpsimd.collective_compute(
                "AllGather", ALU.bypass, ins=[ag3_in.ap()], outs=[ag3_out.ap()],
                replica_groups=[list(range(NC))])
            dhTg_f = sp1.tile([128, 8, B], F32, tag="dhTgf")
            nc.sync.dma_start(dhTg_f[:], ag3_out.ap().rearrange("(c k) n -> k c n", k=128))
            dhTg_sb = sp1.tile([128, 8, B], R32, tag="dhTg")
            nc.vector.tensor_copy(dhTg_sb[:], dhTg_f[:])

        if PL >= 6:
            # ---------- phase 6: heads ----------
            hd_ps = ps_g.tile([B, 82], F32, tag="g")
            nc.tensor.matmul(hd_ps[:], ones_sb[:], bpg_sb[:], start=True, stop=False)
            for c in range(8):
                nc.tensor.matmul(hd_ps[:], dhTg_sb[:, c, :], wpg_sb[:, c, :],
                                 start=False, stop=False)
            for c in range(4):
                nc.tensor.matmul(hd_ps[:], ctxT_sb[:, c, :], wpg_sb[:, 8 + c, :],
                                 start=False, stop=(c == 3))
            hd_sb = sp1.tile([B, 82], F32, tag="hd")
            nc.vector.tensor_copy(hd_sb[:], hd_ps[:])
            nc.sync.dma_start(o_hd.ap(), hd_sb[:])

    nc.compile()
    return nc


_NC_CACHE = None


def _get_nc():
    global _NC_CACHE
    if _NC_CACHE is None:
        _NC_CACHE = _build()
    return _NC_CACHE


def _host_prep(inp):
    f = lambda x: np.ascontiguousarray(np.asarray(inp[x], np.float32))
    pre1T = ((2.0 * _m1)[:, None] * f('W_pre1')).T.copy()
    pre2T = ((2.0 * _m2)[:, None] * f('W_pre2')).T.copy()
    dec_inT = f('decoder_input').T.copy()
    ctxaT = f('attention_context').T.copy()
    hT_att = f('attention_hidden').T.copy()
    hT_dec = f('decoder_hidden').T.copy()
    W_qT = f('W_q').T.copy()
    Wcomb = np.einsum('dc,cej->dej', f('W_loc'), f('W_conv'))
    conv_lhsT = np.ascontiguousarray(Wcomb.transpose(1, 2, 0).reshape(62, 128))
    v_col = f('v_att')[0][:, None].copy()
    WpgT = np.zeros((HID + ENC, 82), np.float32)
    WpgT[:, :81] = np.concatenate([f('W_proj'), f('W_gate')], axis=0).T
    b_pg = np.zeros((1, 82), np.float32)
    b_pg[0, :81] = np.concatenate([f('b_proj'), f('b_gate')])
    ident = np.eye(128, dtype=np.float32)
    awc_pad = np.zeros((B, 2, T + 2 * PAD), np.float32)
    awc_pad[:, 0, PAD:PAD + T] = f('attention_weights')
    awc_pad[:, 1, PAD:PAD + T] = f('attention_weights_cum')
    pm_t = np.ascontiguousarray(f('processed_memory').transpose(2, 0, 1))
    b_att_v = f('bih_att') + f('bhh_att')
    b_dec_v = f('bih_dec') + f('bhh_dec')
    Wih_att, Whh_att = f('Wih_att'), f('Whh_att')
    Wih_dec, Whh_dec = f('Wih_dec'), f('Whh_dec')
    att_cell, dec_cell = f('attention_cell'), f('decoder_cell')
    awcum_full = f('attention_weights_cum')
    memory = np.asarray(inp['memory'], np.float32)

    in_maps = []
    for k in range(NC):
        idx = np.concatenate([np.arange(g * HID + k * 128, g * HID + (k + 1) * 128)
                              for g in range(4)])
        in_maps.append(dict(
            pre1T=pre1T, pre2T=pre2T, dec_inT=dec_inT, ctxaT=ctxaT,
            hT_att=hT_att, hT_dec=hT_dec, W_qT=W_qT, conv_lhsT=conv_lhsT,
            v_col=v_col, WpgT=WpgT, b_pg=b_pg, identF=ident, identR=ident,
            ones_in=np.ones((1, 128), np.float32),
            Wih_attT=Wih_att[idx].T.copy(), Whh_attT=Whh_att[idx].T.copy(),
            b_att=b_att_v[idx][None, :].copy(),
            Wih_decT=Wih_dec[idx].T.copy(), Whh_decT=Whh_dec[idx].T.copy(),
            b_dec=b_dec_v[idx][None, :].copy(),
            bsel=np.eye(B, dtype=np.float32)[:, k * BS:(k + 1) * BS].copy(),
            catt=att_cell[:, k * 128:(k + 1) * 128].copy(),
            cdec=dec_cell[:, k * 128:(k + 1) * 128].copy(),
            awc_pad=awc_pad[k * BS:(k + 1) * BS].copy(),
            awc_in=awcum_full[k * BS:(k + 1) * BS].copy(),
            pm_t=pm_t[:, k * BS:(k + 1) * BS, :].copy(),
            mem=np.ascontiguousarray(memory[k * BS:(k + 1) * BS]),
        ))
    return in_maps


def kernel(**inputs):
    nc = _get_nc()
    in_maps = _host_prep(inputs)
    res = run_bass_kernel_spmd(nc, in_maps, list(range(NC)))
    r = res.results
    dec_out = r[0]['o_hd'][:, :80].copy()
    gate = r[0]['o_hd'][:, 80:81].copy()
    ah = np.concatenate([r[k]['o_ah'] for k in range(NC)], axis=1)
    ac = np.concatenate([r[k]['o_ac'] for k in range(NC)], axis=1)
    dh = np.concatenate([r[k]['o_dh'] for k in range(NC)], axis=1)
    dc = np.concatenate([r[k]['o_dc'] for k in range(NC)], axis=1)
    aw = np.concatenate([r[k]['o_aw'] for k in range(NC)], axis=0)
    awcum = np.concatenate([r[k]['o_awcum'] for k in range(NC)], axis=0)
    ctx = np.concatenate([r[k]['o_ctx'] for k in range(NC)], axis=0)
    return (dec_out, gate, ah, ac, dh, dc, aw, awcum, ctx)
